# revision 1
# baseline (speedup 1.0000x reference)
"""DeepPheno model kernel for 8 TRN2 NeuronCores.

Computation (reference):
    h    = gelu(gos @ W1 + b1)                     (B, HID)     erf-gelu
    x    = concat([h, exp_x], 1)                   (B, HID+EXP)
    flat = sigmoid(x @ W2 + b2)                    (B, C)
    out  = max_i flat[b, j] * M[i, j]              (B, C)

Since flat = sigmoid(..) > 0, the max-pool factorizes exactly:
    out[b, j] = flat[b, j] * max_i M[i, j]
(multiplying by a positive scalar is monotone, so the max over i is attained
at argmax_i M[i, j] on both sides and the products round identically).

Sharding over 8 cores (SPMD, all differences live in the shard data):
  - matmul1 split by HID columns: core c owns hid rows [192c, 192(c+1)) of
    h.T (HID padded 1500 -> 1536 with zero W1 columns, gelu(0)=0).
  - AllGather of the 8 gelu'd (192, 64) chunks -> every core holds the full
    x.T contraction operand for matmul2.
  - matmul2 / b2 / hpo colmax / output split by class columns: core c owns
    classes [256c, 256(c+1)).
Weight tensors are read by exactly one core; only gos (2.5MB) is replicated.

Matmuls run in fp16 (fp32 matmul is 4 cycles/row and never HAM-warms; fp16 is
1 cycle/row): DMA stays fp32 (exact bytes), operands are cast to fp16 on
device overlapped with the DMA stream. PSUM accumulation, bias-add,
activations, colmax and the final multiply stay fp32.

All device tensors are host-prepacked into SBUF-image layout (128, free) so
every DMA moves long contiguous per-partition rows.
"""

import numpy as np

import concourse.bacc as bacc
import concourse.mybir as mybir
import concourse.tile as tile
from concourse.bass_utils import run_bass_kernel_spmd

# Problem shape (hardcoded per contract)
B = 64
IN = 10000
EXP = 53
HID = 1500
C = 2048

NCORES = 8
HD = 192          # hid columns per core (1536 / 8)
HIDP = HD * NCORES
CD = C // NCORES  # 256 classes per core
KT1 = 79          # k tiles for matmul1: 79 * 128 = 10112 >= 10000
K1P = KT1 * 128
KT2 = 13          # k tiles for matmul2: 13 * 128 = 1664 = 1536 + 128
K2P = KT2 * 128

F32 = mybir.dt.float32
F16 = mybir.dt.float16

# DMA chunking (k-tile boundaries)
GOS_CHUNKS = [0, 20, 40, 60, KT1]
W1A_CHUNKS = [0, 16, 32, 48, 64, KT1]   # m-block 0 (128 cols)
W1B_CHUNKS = [0, 27, 53, KT1]           # m-block 1 (64 cols)


def _build_nc():
    nc = bacc.Bacc(
        "TRN2",
        target_bir_lowering=False,
        debug=False,
        enable_asserts=False,
        num_devices=NCORES,
    )

    # External I/O, all in SBUF-image layout (128, free)
    gos_d = nc.dram_tensor("gos_img", [128, KT1 * B], F32, kind="ExternalInput")
    w1a_d = nc.dram_tensor("w1a_img", [128, KT1 * 128], F32, kind="ExternalInput")
    w1b_d = nc.dram_tensor("w1b_img", [128, KT1 * 64], F32, kind="ExternalInput")
    w2_d = nc.dram_tensor("w2_img", [128, 2 * KT2 * 128], F32, kind="ExternalInput")
    exp_d = nc.dram_tensor("exp_img", [128, B], F32, kind="ExternalInput")
    mt_d = nc.dram_tensor("mt_img", [128, 2 * C], F32, kind="ExternalInput")
    b1_d = nc.dram_tensor("b1_img", [128, 2], F32, kind="ExternalInput")
    b2_d = nc.dram_tensor("b2_img", [128, 2], F32, kind="ExternalInput")
    out_d = nc.dram_tensor("out_img", [128, 2 * B], F32, kind="ExternalOutput")

    with tile.TileContext(nc) as tc:
        with (
            tc.tile_pool(name="persist", bufs=1) as pp,
            tc.tile_pool(name="small", bufs=1) as sp,
            tc.tile_pool(name="psum", bufs=1, space="PSUM") as psp,
            tc.tile_pool(name="dram", bufs=1, space="DRAM") as dp,
        ):
            # --- dummy tiny AllGather issued first: absorbs the ncfw entry
            # barrier (~30-50us) while the big DMAs stream, so the real
            # gather later pays only its own latency ---
            dumm_in = dp.tile([1, 8], F32, tag="dumm_in")
            dumm_out = dp.tile([2, 8], F32, tag="dumm_out")
            dumm_sb = sp.tile([1, 8], F32, tag="dumm_sb")
            nc.vector.memset(dumm_sb[:, :], 0.0)
            nc.sync.dma_start(out=dumm_in[:, :], in_=dumm_sb[:, :])
            nc.gpsimd.collective_compute(
                "AllGather",
                mybir.AluOpType.bypass,
                replica_groups=[[2 * g, 2 * g + 1] for g in range(NCORES // 2)],
                ins=[dumm_in[:, :].opt()],
                outs=[dumm_out[:, :].opt()],
            )

            # --- small loads first (keep them off the big-DMA tail) ---
            b1_sb = sp.tile([128, 2], F32, tag="b1")
            nc.sync.dma_start(out=b1_sb[:, :], in_=b1_d[:, :])
            b2_sb = sp.tile([128, 2], F32, tag="b2")
            nc.sync.dma_start(out=b2_sb[:, :], in_=b2_d[:, :])
            exp_sb = sp.tile([128, B], F32, tag="expt")
            nc.sync.dma_start(out=exp_sb[:, :], in_=exp_d[:, :])
            exp16 = sp.tile([128, B], F16, tag="expt16")
            nc.scalar.copy(exp16[:, :], exp_sb[:, :])

            # --- streaming operand gos.T, chunked; cast fp16 on DVE ---
            gos_sb = pp.tile([128, KT1 * B], F32, tag="gos")
            gos16 = pp.tile([128, KT1 * B], F16, tag="gos16")
            for a, b in zip(GOS_CHUNKS[:-1], GOS_CHUNKS[1:]):
                sl = slice(a * B, b * B)
                nc.sync.dma_start(out=gos_sb[:, sl], in_=gos_d[:, sl])
                nc.vector.tensor_copy(gos16[:, sl], gos_sb[:, sl])

            # --- W1 shard, m-block-major, chunked; cast on DVE/ACT alternating ---
            w1a_sb = pp.tile([128, KT1 * 128], F32, tag="w1a")
            w1a16 = pp.tile([128, KT1 * 128], F16, tag="w1a16")
            for i, (a, b) in enumerate(zip(W1A_CHUNKS[:-1], W1A_CHUNKS[1:])):
                sl = slice(a * 128, b * 128)
                nc.sync.dma_start(out=w1a_sb[:, sl], in_=w1a_d[:, sl])
                if i % 2 == 0:
                    nc.vector.tensor_copy(w1a16[:, sl], w1a_sb[:, sl])
                else:
                    nc.scalar.copy(w1a16[:, sl], w1a_sb[:, sl])
            w1b_sb = pp.tile([128, KT1 * 64], F32, tag="w1b")
            w1b16 = pp.tile([128, KT1 * 64], F16, tag="w1b16")
            for i, (a, b) in enumerate(zip(W1B_CHUNKS[:-1], W1B_CHUNKS[1:])):
                sl = slice(a * 64, b * 64)
                nc.sync.dma_start(out=w1b_sb[:, sl], in_=w1b_d[:, sl])
                if i % 2 == 0:
                    nc.vector.tensor_copy(w1b16[:, sl], w1b_sb[:, sl])
                else:
                    nc.scalar.copy(w1b16[:, sl], w1b_sb[:, sl])

            # --- W2 shard (both m-blocks), then hpo M.T shard ---
            w2_sb = pp.tile([128, 2 * KT2 * 128], F32, tag="w2")
            w2_16 = pp.tile([128, 2 * KT2 * 128], F16, tag="w2_16")
            for mb in range(2):
                sl = slice(mb * KT2 * 128, (mb + 1) * KT2 * 128)
                nc.sync.dma_start(out=w2_sb[:, sl], in_=w2_d[:, sl])
                nc.vector.tensor_copy(w2_16[:, sl], w2_sb[:, sl])
            mt_sb = pp.tile([128, 2 * C], F32, tag="mt")
            cm_sb = sp.tile([128, 2], F32, tag="cm")
            for mb in range(2):
                sl = slice(mb * C, (mb + 1) * C)
                nc.sync.dma_start(out=mt_sb[:, sl], in_=mt_d[:, sl])
                nc.vector.reduce_max(
                    cm_sb[:, mb : mb + 1], mt_sb[:, sl], axis=mybir.AxisListType.X
                )

            # --- matmul1: h.T chunk = W1c.T @ gos.T, k accumulated in PSUM ---
            ph0 = psp.tile([128, B], F32, tag="ph0")
            for n in range(KT1):
                nc.tensor.matmul(
                    ph0[:, :],
                    lhsT=w1a16[:, n * 128 : (n + 1) * 128],
                    rhs=gos16[:, n * B : (n + 1) * B],
                    start=(n == 0),
                    stop=(n == KT1 - 1),
                )
            ph1 = psp.tile([64, B], F32, tag="ph1")
            for n in range(KT1):
                nc.tensor.matmul(
                    ph1[:, :],
                    lhsT=w1b16[:, n * 64 : (n + 1) * 64],
                    rhs=gos16[:, n * B : (n + 1) * B],
                    start=(n == 0),
                    stop=(n == KT1 - 1),
                )

            # gelu(pre + b1) (exact erf gelu), output fp16 for the gather
            h0_sb = sp.tile([128, B], F16, tag="h0")
            nc.scalar.activation(
                h0_sb[:, :], ph0[:, :],
                mybir.ActivationFunctionType.Gelu, bias=b1_sb[:, 0:1],
            )
            h1_sb = sp.tile([64, B], F16, tag="h1")
            nc.scalar.activation(
                h1_sb[:, :], ph1[:, :],
                mybir.ActivationFunctionType.Gelu, bias=b1_sb[0:64, 1:2],
            )

            # --- AllGather the (192, B) fp16 chunk -> full (1536, B) h.T ---
            ag_in = dp.tile([HD, B], F16, tag="ag_in")
            nc.sync.dma_start(out=ag_in[0:128, :], in_=h0_sb[:, :])
            nc.sync.dma_start(out=ag_in[128:HD, :], in_=h1_sb[:, :])
            ag_out = dp.tile([HIDP, B], F16, tag="ag_out")
            nc.gpsimd.collective_compute(
                "AllGather",
                mybir.AluOpType.bypass,
                replica_groups=[list(range(NCORES))],
                ins=[ag_in[:, :].opt()],
                outs=[ag_out[:, :].opt()],
            )

            # full x.T (k on partitions): 12 tiles from the gather + exp tile.
            # Two DMAs on the two HWDGE rings (sync + scalar) — same-ring
            # HWDGE DMAs serialize, so one ring alone costs ~2.8us here.
            x_sb = pp.tile([128, (KT2 - 1) * B], F16, tag="xsb")
            for q, eng in enumerate((nc.sync, nc.scalar)):
                t0, t1 = 6 * q, 6 * (q + 1)
                eng.dma_start(
                    out=x_sb[:, t0 * B : t1 * B].rearrange(
                        "p (n m) -> p n m", n=6
                    ),
                    in_=ag_out[t0 * 128 : t1 * 128, :].rearrange(
                        "(n p) m -> p n m", p=128
                    ),
                )

            # --- matmul2: flat.T chunk = W2c.T @ x.T ---
            # The exp k-tile doesn't depend on the gather: run it first
            # (start=True) so it executes during the collective window.
            pf0 = psp.tile([128, B], F32, tag="pf0")
            pf1 = psp.tile([128, B], F32, tag="pf1")
            for mb, pf in ((0, pf0), (1, pf1)):
                base = mb * KT2 * 128
                for t in [KT2 - 1] + list(range(KT2 - 1)):
                    rhs = (
                        x_sb[:, t * B : (t + 1) * B]
                        if t < KT2 - 1
                        else exp16[:, :]
                    )
                    nc.tensor.matmul(
                        pf[:, :],
                        lhsT=w2_16[:, base + t * 128 : base + (t + 1) * 128],
                        rhs=rhs,
                        start=(t == KT2 - 1),
                        stop=(t == KT2 - 2),
                    )

            # sigmoid(pre + b2)
            f0_sb = sp.tile([128, B], F32, tag="f0")
            nc.scalar.activation(
                f0_sb[:, :], pf0[:, :],
                mybir.ActivationFunctionType.Sigmoid, bias=b2_sb[:, 0:1],
            )
            f1_sb = sp.tile([128, B], F32, tag="f1")
            nc.scalar.activation(
                f1_sb[:, :], pf1[:, :],
                mybir.ActivationFunctionType.Sigmoid, bias=b2_sb[:, 1:2],
            )

            # out.T chunk = flat.T * colmax: per-partition scale via the ACT
            # Copy path (same engine as the sigmoid — no cross-engine hop);
            # two output DMAs on separate HWDGE rings.
            o_sb = sp.tile([128, 2 * B], F32, tag="osb")
            nc.scalar.mul(o_sb[:, :B], f0_sb[:, :], cm_sb[:, 0:1])
            nc.sync.dma_start(out=out_d[:, :B], in_=o_sb[:, :B])
            nc.scalar.mul(o_sb[:, B:], f1_sb[:, :], cm_sb[:, 1:2])
            nc.scalar.dma_start(out=out_d[:, B:], in_=o_sb[:, B:])

    nc.compile()
    return nc


_NC_CACHE = None


def _get_nc():
    global _NC_CACHE
    if _NC_CACHE is None:
        _NC_CACHE = _build_nc()
    return _NC_CACHE


def _tile_img(arr2d, ktiles):
    """(ktiles*128, m) -> SBUF image (128, ktiles*m), k-tile-major free dim."""
    k, m = arr2d.shape
    assert k == ktiles * 128
    return np.ascontiguousarray(
        arr2d.reshape(ktiles, 128, m).transpose(1, 0, 2).reshape(128, ktiles * m)
    )


def _prep_inputs(gos, exp_x, W1, b1, W2, b2, hpo_matrix):
    f = np.float32
    gos = np.asarray(gos, f)
    exp_x = np.asarray(exp_x, f)
    W1 = np.asarray(W1, f)
    b1 = np.asarray(b1, f)
    W2 = np.asarray(W2, f)
    b2 = np.asarray(b2, f)
    M = np.asarray(hpo_matrix, f)

    # gos.T padded to K1P rows, shared across cores
    gosT = np.zeros((K1P, B), f)
    gosT[:IN] = gos.T
    gos_img = _tile_img(gosT, KT1)

    # exp_x.T padded to one 128-row k-tile, shared
    expT = np.zeros((128, B), f)
    expT[:EXP] = exp_x.T
    exp_img = np.ascontiguousarray(expT)

    # W1 padded to (K1P, HIDP)
    W1p = np.zeros((K1P, HIDP), f)
    W1p[:IN, :HID] = W1
    b1p = np.zeros((HIDP,), f)
    b1p[:HID] = b1

    # W2 rows remapped to x.T layout [h(0:1536) | exp(1536:1589) | 0 pad]
    W2p = np.zeros((K2P, C), f)
    W2p[:HID] = W2[:HID]
    W2p[HIDP : HIDP + EXP] = W2[HID:]

    in_maps = []
    for c in range(NCORES):
        h0, h1 = HD * c, HD * (c + 1)
        c0, c1 = CD * c, CD * (c + 1)

        w1a_img = _tile_img(W1p[:, h0 : h0 + 128], KT1)
        w1b_img = _tile_img(W1p[:, h0 + 128 : h1], KT1)

        w2c = W2p[:, c0:c1]
        w2_img = np.concatenate(
            [_tile_img(w2c[:, :128], KT2), _tile_img(w2c[:, 128:], KT2)], axis=1
        )

        mt = np.ascontiguousarray(M[:, c0:c1].T)  # (256, 2048)
        mt_img = np.concatenate([mt[:128], mt[128:]], axis=1)  # (128, 2C)

        b1_img = np.zeros((128, 2), f)
        b1_img[:, 0] = b1p[h0 : h0 + 128]
        b1_img[:64, 1] = b1p[h0 + 128 : h1]
        b2_img = np.zeros((128, 2), f)
        b2_img[:, 0] = b2[c0 : c0 + 128]
        b2_img[:, 1] = b2[c0 + 128 : c1]

        in_maps.append(
            {
                "gos_img": gos_img,
                "w1a_img": np.ascontiguousarray(w1a_img),
                "w1b_img": np.ascontiguousarray(w1b_img),
                "w2_img": np.ascontiguousarray(w2_img),
                "exp_img": exp_img,
                "mt_img": np.ascontiguousarray(mt_img),
                "b1_img": b1_img,
                "b2_img": b2_img,
            }
        )
    return in_maps


def _assemble_output(results):
    cols = []
    for r in results:
        o = r["out_img"]  # (128, 2B): [p, t*B + b] = flat.T[t*128+p, b] * cm
        chunk = o.reshape(128, 2, B).transpose(1, 0, 2).reshape(CD, B)
        cols.append(chunk.T)  # (B, CD)
    return np.ascontiguousarray(np.concatenate(cols, axis=1))


def kernel(gos, exp_x, W1, b1, W2, b2, hpo_matrix, **kw):
    nc = _get_nc()
    in_maps = _prep_inputs(gos, exp_x, W1, b1, W2, b2, hpo_matrix)
    res = run_bass_kernel_spmd(nc, in_maps, core_ids=list(range(NCORES)))
    return _assemble_output(res.results)



# revision 3
# speedup vs baseline: 1.5439x; 1.5439x over previous
"""DeepPheno model kernel for 8 TRN2 NeuronCores — collective-free design.

Computation (reference):
    h    = gelu(gos @ W1 + b1)                     (B, HID)     erf-gelu
    x    = concat([h, exp_x], 1)                   (B, HID+EXP)
    flat = sigmoid(x @ W2 + b2)                    (B, C)
    out  = max_i flat[b, j] * M[i, j]              (B, C)

Since flat = sigmoid(..) > 0, the max-pool factorizes exactly:
    out[b, j] = flat[b, j] * max_i M[i, j]

Why no collectives: on this stack the first collective of an execution
pays a ~54us ncfw entry barrier plus ~30us+ of trigger->data latency, a
~90us serial chain that dominates the whole kernel (the previous sharded
design sat at ~95-110us because of it). Instead every core redundantly
computes matmul1 from the FULL W1, which is affordable because W1 is
carried in fp8e4m3 (host-cast): 15.7MB/core streams at ~354GB/s in ~44us,
fully overlapped with the matmul1 that consumes it.

Sharding: matmul1 fully replicated; W2 / b2 / hpo colmax / output are
split by class columns (core c owns classes [256c, 256(c+1))).

matmul1 runs "flipped" (h, not h.T): the tiny gos tile (128, 2, 64) is
the stationary operand (so the 15.7MB W1 stream pays no LDWEIGHTS) and
W1 streams as the moving operand in N=512 fp8 DoubleRow matmuls
(0.5 cycles/row, 256 contraction rows per instruction).

Precision: W1 is scaled by 64 on host before the e4m3 cast (raw W1
values ~N(0, 0.01) sit below e4m3's min normal 2^-6; scaling moves them
into the normal range; the gelu undoes it with scale=1/64). gos is cast
to e4m3 unscaled (values in [0,1)). Everything downstream is fp16/fp32:
h fp16, W2/exp fp16 (standard-mode matmul2, fp32 PSUM), sigmoid/colmax
multiply fp32, hpo matrix fp16 for the colmax. Measured rel_l2 ~6e-3
against the fp32 reference (gate 2e-2).

b1/b2 are folded into the matmuls: one zero-pad row of gos.T / x.T is
set to 1.0 and the matching W1 / W2 row carries the bias vector.
"""

import numpy as np
import ml_dtypes

import concourse.bacc as bacc
import concourse.mybir as mybir
import concourse.tile as tile
from concourse.bass_utils import run_bass_kernel_spmd
from concourse.masks import make_identity

# Problem shape (hardcoded per contract)
B = 64
IN = 10000
EXP = 53
HID = 1500
C = 2048

NCORES = 8
CD = C // NCORES        # 256 classes per core
KT1 = 80                # k tiles for matmul1: 80 * 128 = 10240 >= 10000 (even)
K1P = KT1 * 128
NBLK = 3                # matmul1 output blocks of 512 (3 * 512 = 1536 >= 1500)
HIDP = NBLK * 512
KT2 = 13                # k tiles for matmul2: 12 h-tiles + 1 exp/bias tile
K2P = KT2 * 128
W1SCALE = 64.0          # power of two; moves W1 into e4m3 normal range

F32 = mybir.dt.float32
F16 = mybir.dt.float16
F8 = mybir.dt.float8e4  # ml_dtypes.float8_e4m3

W1_CHUNK = 10           # k-tiles per W1 DMA chunk (must be even; 8 chunks/block)


def _build_nc():
    nc = bacc.Bacc(
        "TRN2",
        target_bir_lowering=False,
        debug=False,
        enable_asserts=False,
        num_devices=NCORES,
    )

    # External I/O, all in SBUF-image layout (128, free)
    w1_d = nc.dram_tensor("w1_img", [128, NBLK * KT1 * 512], F8, kind="ExternalInput")
    gos_d = nc.dram_tensor("gos_img", [128, KT1 * B], F8, kind="ExternalInput")
    w2_d = nc.dram_tensor("w2_img", [128, KT2 * CD], F16, kind="ExternalInput")
    exp_d = nc.dram_tensor("exp_img", [128, B], F16, kind="ExternalInput")
    mt_d = nc.dram_tensor("mt_img", [128, 2 * C], F16, kind="ExternalInput")
    out_d = nc.dram_tensor("out_img", [128, 2 * B], F32, kind="ExternalOutput")

    with tile.TileContext(nc) as tc:
        with (
            tc.tile_pool(name="big", bufs=1) as pp,
            tc.tile_pool(name="small", bufs=1) as sp,
            tc.tile_pool(name="ph", bufs=1, space="PSUM") as php,
            tc.tile_pool(name="pt", bufs=3, space="PSUM") as ptp,
            tc.tile_pool(name="pf", bufs=1, space="PSUM") as pfp,
        ):
            # identity for PE transposes (built on gpsimd, no DMA)
            ident = sp.tile([B, B], F16, tag="ident")
            make_identity(nc, ident[:, :])

            # --- small loads on the scalar HWDGE ring (Q10), off the W1 path
            exp_sb = sp.tile([128, B], F16, tag="exp")
            nc.scalar.dma_start(out=exp_sb[:, :], in_=exp_d[:, :])
            w2_sb = sp.tile([128, KT2 * CD], F16, tag="w2")
            nc.scalar.dma_start(out=w2_sb[:, :], in_=w2_d[:, :])
            mt_sb = pp.tile([128, 2 * C], F16, tag="mt")
            cm_sb = sp.tile([128, 2], F32, tag="cm")
            for cb in range(2):
                sl = slice(cb * C, (cb + 1) * C)
                nc.scalar.dma_start(out=mt_sb[:, sl], in_=mt_d[:, sl])
                nc.vector.reduce_max(
                    cm_sb[:, cb : cb + 1], mt_sb[:, sl], axis=mybir.AxisListType.X
                )

            # --- gos (stationary operand) first on the sync ring (Q1)
            gos_sb = pp.tile([128, KT1 * B], F8, tag="gos")
            nc.sync.dma_start(out=gos_sb[:, :], in_=gos_d[:, :])

            # --- W1 streamed block-major; matmul1 consumes chunk by chunk
            w1_sb = pp.tile([128, NBLK * KT1 * 512], F8, tag="w1")
            h_sb = sp.tile([B, HIDP], F16, tag="h")
            xT_sb = sp.tile([128, KT2 * B], F16, tag="xT")
            psum_h = [
                php.tile([B, 512], F32, tag=f"ph{nb}", name=f"ph{nb}")
                for nb in range(NBLK)
            ]
            psum_f = [
                pfp.tile([128, B], F32, tag=f"pf{cb}", name=f"pf{cb}")
                for cb in range(2)
            ]

            def mm2(cb, kt, start, stop):
                w2sl = w2_sb[:, kt * CD + cb * 128 : kt * CD + cb * 128 + 128]
                nc.tensor.matmul(
                    psum_f[cb][:, :],
                    lhsT=w2sl,
                    rhs=(exp_sb[:, :] if kt == KT2 - 1 else xT_sb[:, kt * B : (kt + 1) * B]),
                    start=start,
                    stop=stop,
                )

            for nb in range(NBLK):
                base = nb * KT1 * 512
                for ci in range(0, KT1, W1_CHUNK):
                    sl = slice(base + ci * 512, base + (ci + W1_CHUNK) * 512)
                    nc.sync.dma_start(out=w1_sb[:, sl], in_=w1_d[:, sl])
                for t in range(KT1 // 2):
                    nc.tensor.matmul(
                        psum_h[nb][:, :],
                        lhsT=gos_sb[:, 2 * t * B : (2 * t + 2) * B].rearrange(
                            "p (k b) -> p k b", k=2
                        ),
                        rhs=w1_sb[
                            :, base + 2 * t * 512 : base + (2 * t + 2) * 512
                        ].rearrange("p (k f) -> p k f", k=2),
                        start=(t == 0),
                        stop=(t == KT1 // 2 - 1),
                        perf_mode=mybir.MatmulPerfMode.DoubleRow,
                    )
                if nb == 0:
                    # exp/bias k-tile of matmul2 opens the psum_f groups; its
                    # operands arrive early on Q10, and the PE reaches this
                    # point long before the h transposes are ready.
                    for cb in range(2):
                        mm2(cb, KT2 - 1, start=True, stop=False)
                # gelu undoes the host-side W1 scaling; erf gelu
                nc.scalar.activation(
                    h_sb[:, nb * 512 : (nb + 1) * 512],
                    psum_h[nb][:, :],
                    mybir.ActivationFunctionType.Gelu,
                    scale=1.0 / W1SCALE,
                )
                # transpose the 4 fresh h k-tiles and feed matmul2
                for q in range(4):
                    kt = nb * 4 + q
                    pt = ptp.tile([128, B], F16, tag="pt")
                    nc.tensor.transpose(
                        pt[:, :], h_sb[:, kt * 128 : (kt + 1) * 128], ident[:, :]
                    )
                    nc.vector.tensor_copy(xT_sb[:, kt * B : (kt + 1) * B], pt[:, :])
                for cb in range(2):
                    for q in range(4):
                        kt = nb * 4 + q
                        mm2(cb, kt, start=False, stop=(nb == NBLK - 1 and q == 3))

            # sigmoid(pre) then the colmax scale; outputs split over both rings
            f_sb = sp.tile([128, 2 * B], F32, tag="f")
            o_sb = sp.tile([128, 2 * B], F32, tag="o")
            for cb in range(2):
                nc.scalar.activation(
                    f_sb[:, cb * B : (cb + 1) * B],
                    psum_f[cb][:, :],
                    mybir.ActivationFunctionType.Sigmoid,
                )
                nc.scalar.mul(
                    o_sb[:, cb * B : (cb + 1) * B],
                    f_sb[:, cb * B : (cb + 1) * B],
                    cm_sb[:, cb : cb + 1],
                )
            nc.sync.dma_start(out=out_d[:, 0:B], in_=o_sb[:, 0:B])
            nc.scalar.dma_start(out=out_d[:, B : 2 * B], in_=o_sb[:, B : 2 * B])

    nc.compile()
    return nc


_NC_CACHE = None


def _get_nc():
    global _NC_CACHE
    if _NC_CACHE is None:
        _NC_CACHE = _build_nc()
    return _NC_CACHE


def _prep_inputs(gos, exp_x, W1, b1, W2, b2, hpo_matrix):
    f = np.float32
    gos = np.asarray(gos, f)
    exp_x = np.asarray(exp_x, f)
    W1 = np.asarray(W1, f)
    b1 = np.asarray(b1, f)
    W2 = np.asarray(W2, f)
    b2 = np.asarray(b2, f)
    M = np.asarray(hpo_matrix, f)
    f8 = ml_dtypes.float8_e4m3

    # W1 padded to (K1P, HIDP); bias row at K1P-1 pairs with the gos ones-row
    W1p = np.zeros((K1P, HIDP), f)
    W1p[:IN, :HID] = W1
    W1p[K1P - 1, :HID] = b1
    w1_img = np.ascontiguousarray(
        (W1p * W1SCALE)
        .astype(f8)
        .reshape(KT1, 128, NBLK, 512)
        .transpose(1, 2, 0, 3)
        .reshape(128, NBLK * KT1 * 512)
    )

    # gos.T padded to K1P rows with the ones-row last (b1 fold)
    gosT = np.zeros((K1P, B), f)
    gosT[:IN] = gos.T
    gosT[K1P - 1] = 1.0
    gos_img = np.ascontiguousarray(
        gosT.astype(f8).reshape(KT1, 128, B).transpose(1, 0, 2).reshape(128, KT1 * B)
    )

    # exp/bias k-tile of x.T: rows 0..52 exp.T, row 53 ones (b2 fold)
    exp_img = np.zeros((128, B), np.float16)
    exp_img[:EXP] = exp_x.T.astype(np.float16)
    exp_img[EXP] = 1.0

    # W2 rows remapped to x.T layout [h(0:1536) | exp(1536:1589) | b2 row]
    W2p = np.zeros((K2P, C), f)
    W2p[:HID] = W2[:HID]
    W2p[HIDP : HIDP + EXP] = W2[HID:]
    W2p[HIDP + EXP] = b2
    W2p16 = W2p.astype(np.float16)

    in_maps = []
    for c in range(NCORES):
        c0 = CD * c
        w2_img = np.ascontiguousarray(
            W2p16[:, c0 : c0 + CD]
            .reshape(KT2, 128, CD)
            .transpose(1, 0, 2)
            .reshape(128, KT2 * CD)
        )
        mt = M[:, c0 : c0 + CD].T.astype(np.float16)  # (256, 2048)
        mt_img = np.ascontiguousarray(np.concatenate([mt[:128], mt[128:]], axis=1))
        in_maps.append(
            {
                "w1_img": w1_img,
                "gos_img": gos_img,
                "w2_img": w2_img,
                "exp_img": exp_img,
                "mt_img": mt_img,
            }
        )
    return in_maps


def _assemble_output(results):
    cols = []
    for r in results:
        o = r["out_img"]  # (128, 2B): [p, cb*B + b] = out[b, c0 + cb*128 + p]
        chunk = o.reshape(128, 2, B).transpose(1, 0, 2).reshape(CD, B)
        cols.append(chunk.T)  # (B, CD)
    return np.ascontiguousarray(np.concatenate(cols, axis=1))


def kernel(gos, exp_x, W1, b1, W2, b2, hpo_matrix, **kw):
    nc = _get_nc()
    in_maps = _prep_inputs(gos, exp_x, W1, b1, W2, b2, hpo_matrix)
    res = run_bass_kernel_spmd(nc, in_maps, core_ids=list(range(NCORES)))
    return _assemble_output(res.results)


# revision 8
# speedup vs baseline: 1.5574x; 1.0088x over previous
"""DeepPheno model kernel for 8 TRN2 NeuronCores — collective-free design.

Computation (reference):
    h    = gelu(gos @ W1 + b1)                     (B, HID)     erf-gelu
    x    = concat([h, exp_x], 1)                   (B, HID+EXP)
    flat = sigmoid(x @ W2 + b2)                    (B, C)
    out  = max_i flat[b, j] * M[i, j]              (B, C)

Since flat = sigmoid(..) > 0, the max-pool factorizes exactly:
    out[b, j] = flat[b, j] * max_i M[i, j]

Why no collectives: on this stack the first collective of an execution
pays a ~54us ncfw entry barrier plus ~30us+ of trigger->data latency, a
~90us serial chain that dominates the whole kernel (the previous sharded
design sat at ~95-110us because of it). Instead every core redundantly
computes matmul1 from the FULL W1, which is affordable because W1 is
carried in fp8e4m3 (host-cast): 15.7MB/core streams at ~354GB/s in ~44us,
fully overlapped with the matmul1 that consumes it.

Sharding: matmul1 fully replicated; W2 / b2 / hpo colmax / output are
split by class columns (core c owns classes [256c, 256(c+1))).

matmul1 runs "flipped" (h, not h.T): the tiny gos tile (128, 2, 64) is
the stationary operand (so the 15.7MB W1 stream pays no LDWEIGHTS) and
W1 streams as the moving operand in N=512 fp8 DoubleRow matmuls
(0.5 cycles/row, 256 contraction rows per instruction).

Precision: W1 is scaled by 64 on host before the e4m3 cast (raw W1
values ~N(0, 0.01) sit below e4m3's min normal 2^-6; scaling moves them
into the normal range; the gelu undoes it with scale=1/64). gos is cast
to e4m3 unscaled (values in [0,1)). Everything downstream is fp16/fp32:
h fp16, W2/exp fp16 (standard-mode matmul2, fp32 PSUM), sigmoid/colmax
multiply fp32, hpo matrix fp16 for the colmax. Measured rel_l2 ~6e-3
against the fp32 reference (gate 2e-2).

b1/b2 are folded into the matmuls: one zero-pad row of gos.T / x.T is
set to 1.0 and the matching W1 / W2 row carries the bias vector.
"""

import numpy as np
import ml_dtypes

import concourse.bacc as bacc
import concourse.mybir as mybir
import concourse.tile as tile
from concourse.bass_utils import run_bass_kernel_spmd
from concourse.masks import make_identity

# Problem shape (hardcoded per contract)
B = 64
IN = 10000
EXP = 53
HID = 1500
C = 2048

NCORES = 8
CD = C // NCORES        # 256 classes per core
KT1 = 80                # k tiles for matmul1: 80 * 128 = 10240 >= 10000 (even)
K1P = KT1 * 128
NBLK = 6                # matmul1 output blocks of 256 (6 * 256 = 1536 >= 1500)
BLKW = 256
HIDP = NBLK * BLKW
KT2 = 13                # k tiles for matmul2: 12 h-tiles + 1 exp/bias tile
K2P = KT2 * 128
W1SCALE = 64.0          # power of two; moves W1 into e4m3 normal range

F32 = mybir.dt.float32
F16 = mybir.dt.float16
F8 = mybir.dt.float8e4  # ml_dtypes.float8_e4m3

# k-tile DMA chunking per block (even sizes; final chunks small so the
# last-byte -> last-matmul catch-up is short)
W1_CHUNKS = [[40, 40]] * (NBLK - 1) + [[40, 26, 10, 4]]


def _build_nc():
    nc = bacc.Bacc(
        "TRN2",
        target_bir_lowering=False,
        debug=False,
        enable_asserts=False,
        num_devices=NCORES,
    )

    # External I/O, all in SBUF-image layout (128, free)
    w1_d = nc.dram_tensor("w1_img", [128, NBLK * KT1 * BLKW], F8, kind="ExternalInput")
    gos_d = nc.dram_tensor("gos_img", [128, KT1 * B], F8, kind="ExternalInput")
    w2_d = nc.dram_tensor("w2_img", [128, KT2 * CD], F16, kind="ExternalInput")
    exp_d = nc.dram_tensor("exp_img", [128, B], F16, kind="ExternalInput")
    mt_d = nc.dram_tensor("mt_img", [128, 2 * C], F16, kind="ExternalInput")
    out_d = nc.dram_tensor("out_img", [128, 2 * B], F32, kind="ExternalOutput")

    with tile.TileContext(nc) as tc:
        with (
            tc.tile_pool(name="big", bufs=1) as pp,
            tc.tile_pool(name="small", bufs=1) as sp,
            tc.tile_pool(name="ph", bufs=3, space="PSUM") as php,
            tc.tile_pool(name="pt", bufs=2, space="PSUM") as ptp,
            tc.tile_pool(name="pf", bufs=1, space="PSUM") as pfp,
        ):
            # identity for PE transposes (built on gpsimd, no DMA)
            ident = sp.tile([B, B], F16, tag="ident")
            make_identity(nc, ident[:, :])

            # --- small loads on the scalar HWDGE ring (Q10), off the W1 path
            exp_sb = sp.tile([128, B], F16, tag="exp")
            nc.scalar.dma_start(out=exp_sb[:, :], in_=exp_d[:, :])
            w2_sb = sp.tile([128, KT2 * CD], F16, tag="w2")
            nc.scalar.dma_start(out=w2_sb[:, :], in_=w2_d[:, :])
            mt_sb = pp.tile([128, 2 * C], F16, tag="mt")
            cm_sb = sp.tile([128, 2], F32, tag="cm")
            for cb in range(2):
                sl = slice(cb * C, (cb + 1) * C)
                nc.scalar.dma_start(out=mt_sb[:, sl], in_=mt_d[:, sl])
                nc.vector.reduce_max(
                    cm_sb[:, cb : cb + 1], mt_sb[:, sl], axis=mybir.AxisListType.X
                )

            # --- gos (stationary operand) first on the sync ring (Q1)
            gos_sb = pp.tile([128, KT1 * B], F8, tag="gos")
            nc.sync.dma_start(out=gos_sb[:, :], in_=gos_d[:, :])

            # --- W1 streamed block-major; matmul1 consumes chunk by chunk
            w1_sb = pp.tile([128, NBLK * KT1 * BLKW], F8, tag="w1")
            h_sb = sp.tile([B, HIDP], F16, tag="h")
            xT_sb = sp.tile([128, KT2 * B], F16, tag="xT")
            psum_f = [
                pfp.tile([128, B], F32, tag=f"pf{cb}", name=f"pf{cb}")
                for cb in range(2)
            ]

            def mm2(cb, kt, start, stop):
                w2sl = w2_sb[:, kt * CD + cb * 128 : kt * CD + cb * 128 + 128]
                nc.tensor.matmul(
                    psum_f[cb][:, :],
                    lhsT=w2sl,
                    rhs=(exp_sb[:, :] if kt == KT2 - 1 else xT_sb[:, kt * B : (kt + 1) * B]),
                    start=start,
                    stop=stop,
                )

            # all W1 DMAs up front (one queue, in stream order)
            for nb in range(NBLK):
                base = nb * KT1 * BLKW
                ci = 0
                for ch in W1_CHUNKS[nb]:
                    sl = slice(base + ci * BLKW, base + (ci + ch) * BLKW)
                    nc.sync.dma_start(out=w1_sb[:, sl], in_=w1_d[:, sl])
                    ci += ch

            def mm1_block(nb):
                base = nb * KT1 * BLKW
                psh = php.tile([B, BLKW], F32, tag="ph", name="ph")
                for t in range(KT1 // 2):
                    nc.tensor.matmul(
                        psh[:, :],
                        lhsT=gos_sb[:, 2 * t * B : (2 * t + 2) * B].rearrange(
                            "p (k b) -> p k b", k=2
                        ),
                        rhs=w1_sb[
                            :, base + 2 * t * BLKW : base + (2 * t + 2) * BLKW
                        ].rearrange("p (k f) -> p k f", k=2),
                        start=(t == 0),
                        stop=(t == KT1 // 2 - 1),
                        perf_mode=mybir.MatmulPerfMode.DoubleRow,
                    )
                return psh

            def block_tail(nb, psh):
                # gelu undoes the host-side W1 scaling; erf gelu
                nc.scalar.activation(
                    h_sb[:, nb * BLKW : (nb + 1) * BLKW],
                    psh[:, :],
                    mybir.ActivationFunctionType.Gelu,
                    scale=1.0 / W1SCALE,
                )
                # transpose the 2 fresh h k-tiles and feed matmul2
                for q in range(2):
                    kt = nb * 2 + q
                    pt = ptp.tile([128, B], F16, tag="pt")
                    nc.tensor.transpose(
                        pt[:, :], h_sb[:, kt * 128 : (kt + 1) * 128], ident[:, :]
                    )
                    nc.vector.tensor_copy(xT_sb[:, kt * B : (kt + 1) * B], pt[:, :])
                for cb in range(2):
                    for q in range(2):
                        kt = nb * 2 + q
                        mm2(cb, kt, start=False, stop=(nb == NBLK - 1 and q == 1))

            # PE issue order: block nb's gelu/transpose/mm2 tail is queued
            # AFTER block nb+1's matmul1 stream, so the in-order PE queue
            # never stalls on the ACT engine mid-stream.
            prev = mm1_block(0)
            for cb in range(2):
                # exp/bias k-tile opens the psum_f groups; operands arrive
                # early on Q10, long before the first h transposes.
                mm2(cb, KT2 - 1, start=True, stop=False)
            for nb in range(1, NBLK):
                cur = mm1_block(nb)
                block_tail(nb - 1, prev)
                prev = cur
            block_tail(NBLK - 1, prev)

            # sigmoid(pre) then the colmax scale; outputs split over both rings
            f_sb = sp.tile([128, 2 * B], F32, tag="f")
            o_sb = sp.tile([128, 2 * B], F32, tag="o")
            for cb in range(2):
                nc.scalar.activation(
                    f_sb[:, cb * B : (cb + 1) * B],
                    psum_f[cb][:, :],
                    mybir.ActivationFunctionType.Sigmoid,
                )
                nc.scalar.mul(
                    o_sb[:, cb * B : (cb + 1) * B],
                    f_sb[:, cb * B : (cb + 1) * B],
                    cm_sb[:, cb : cb + 1],
                )
                (nc.sync if cb == 0 else nc.scalar).dma_start(
                    out=out_d[:, cb * B : (cb + 1) * B],
                    in_=o_sb[:, cb * B : (cb + 1) * B],
                )

    nc.compile()
    return nc


_NC_CACHE = None


def _get_nc():
    global _NC_CACHE
    if _NC_CACHE is None:
        _NC_CACHE = _build_nc()
    return _NC_CACHE


def _prep_inputs(gos, exp_x, W1, b1, W2, b2, hpo_matrix):
    f = np.float32
    gos = np.asarray(gos, f)
    exp_x = np.asarray(exp_x, f)
    W1 = np.asarray(W1, f)
    b1 = np.asarray(b1, f)
    W2 = np.asarray(W2, f)
    b2 = np.asarray(b2, f)
    M = np.asarray(hpo_matrix, f)
    f8 = ml_dtypes.float8_e4m3

    # W1 padded to (K1P, HIDP); bias row at K1P-1 pairs with the gos ones-row
    W1p = np.zeros((K1P, HIDP), f)
    W1p[:IN, :HID] = W1
    W1p[K1P - 1, :HID] = b1
    w1_img = np.ascontiguousarray(
        (W1p * W1SCALE)
        .astype(f8)
        .reshape(KT1, 128, NBLK, BLKW)
        .transpose(1, 2, 0, 3)
        .reshape(128, NBLK * KT1 * BLKW)
    )

    # gos.T padded to K1P rows with the ones-row last (b1 fold)
    gosT = np.zeros((K1P, B), f)
    gosT[:IN] = gos.T
    gosT[K1P - 1] = 1.0
    gos_img = np.ascontiguousarray(
        gosT.astype(f8).reshape(KT1, 128, B).transpose(1, 0, 2).reshape(128, KT1 * B)
    )

    # exp/bias k-tile of x.T: rows 0..52 exp.T, row 53 ones (b2 fold)
    exp_img = np.zeros((128, B), np.float16)
    exp_img[:EXP] = exp_x.T.astype(np.float16)
    exp_img[EXP] = 1.0

    # W2 rows remapped to x.T layout [h(0:1536) | exp(1536:1589) | b2 row]
    W2p = np.zeros((K2P, C), f)
    W2p[:HID] = W2[:HID]
    W2p[HIDP : HIDP + EXP] = W2[HID:]
    W2p[HIDP + EXP] = b2
    W2p16 = W2p.astype(np.float16)

    in_maps = []
    for c in range(NCORES):
        c0 = CD * c
        w2_img = np.ascontiguousarray(
            W2p16[:, c0 : c0 + CD]
            .reshape(KT2, 128, CD)
            .transpose(1, 0, 2)
            .reshape(128, KT2 * CD)
        )
        mt = M[:, c0 : c0 + CD].T.astype(np.float16)  # (256, 2048)
        mt_img = np.ascontiguousarray(np.concatenate([mt[:128], mt[128:]], axis=1))
        in_maps.append(
            {
                "w1_img": w1_img,
                "gos_img": gos_img,
                "w2_img": w2_img,
                "exp_img": exp_img,
                "mt_img": mt_img,
            }
        )
    return in_maps


def _assemble_output(results):
    cols = []
    for r in results:
        o = r["out_img"]  # (128, 2B): [p, cb*B + b] = out[b, c0 + cb*128 + p]
        chunk = o.reshape(128, 2, B).transpose(1, 0, 2).reshape(CD, B)
        cols.append(chunk.T)  # (B, CD)
    return np.ascontiguousarray(np.concatenate(cols, axis=1))


def kernel(gos, exp_x, W1, b1, W2, b2, hpo_matrix, **kw):
    nc = _get_nc()
    in_maps = _prep_inputs(gos, exp_x, W1, b1, W2, b2, hpo_matrix)
    res = run_bass_kernel_spmd(nc, in_maps, core_ids=list(range(NCORES)))
    return _assemble_output(res.results)


# revision 16
# speedup vs baseline: 1.6981x; 1.0903x over previous
"""DeepPheno model kernel for 8 TRN2 NeuronCores — collective-free design.

Computation (reference):
    h    = gelu(gos @ W1 + b1)                     (B, HID)     erf-gelu
    x    = concat([h, exp_x], 1)                   (B, HID+EXP)
    flat = sigmoid(x @ W2 + b2)                    (B, C)
    out  = max_i flat[b, j] * M[i, j]              (B, C)

Since flat = sigmoid(..) > 0, the max-pool factorizes exactly:
    out[b, j] = flat[b, j] * max_i M[i, j]

Why no collectives: on this stack the first collective of an execution
pays a ~54us ncfw entry barrier plus ~30us+ of trigger->data latency, a
~90us serial chain that dominates the whole kernel (the previous sharded
design sat at ~95-110us because of it). Instead every core redundantly
computes matmul1 from the FULL W1, which is affordable because W1 is
carried in fp8e4m3 (host-cast): 15.7MB/core streams at ~354GB/s in ~44us,
fully overlapped with the matmul1 that consumes it.

Sharding: matmul1 fully replicated; W2 / b2 / hpo colmax / output are
split by class columns (core c owns classes [256c, 256(c+1))).

matmul1 runs "flipped" (h, not h.T): the tiny gos tile (128, 2, 64) is
the stationary operand (so the 15.7MB W1 stream pays no LDWEIGHTS) and
W1 streams as the moving operand in N=512 fp8 DoubleRow matmuls
(0.5 cycles/row, 256 contraction rows per instruction).

Precision: W1 is scaled by 64 on host before the e4m3 cast (raw W1
values ~N(0, 0.01) sit below e4m3's min normal 2^-6; scaling moves them
into the normal range; the gelu undoes it with scale=1/64). gos is cast
to e4m3 unscaled (values in [0,1)). Everything downstream is fp16/fp32:
h fp16, W2/exp fp16 (standard-mode matmul2, fp32 PSUM), sigmoid/colmax
multiply fp32, hpo matrix fp16 for the colmax. Measured rel_l2 ~6e-3
against the fp32 reference (gate 2e-2).

b1/b2 are folded into the matmuls: one zero-pad row of gos.T / x.T is
set to 1.0 and the matching W1 / W2 row carries the bias vector.
"""

import numpy as np
import ml_dtypes

import concourse.bacc as bacc
import concourse.mybir as mybir
import concourse.tile as tile
from concourse.bass_utils import run_bass_kernel_spmd
from concourse.masks import make_identity

# Problem shape (hardcoded per contract)
B = 64
IN = 10000
EXP = 53
HID = 1500
C = 2048

NCORES = 8
CD = C // NCORES        # 256 classes per core
KT1 = 80                # k tiles for matmul1: 80 * 128 = 10240 >= 10000 (even)
K1P = KT1 * 128
HIDP = HID              # no hid padding: blocks of 256 plus a ragged 220 tail
BLK_W = [256, 256, 256, 256, 256, 220]
BLK_OFF = [0, 256, 512, 768, 1024, 1280]
NBLK = len(BLK_W)
KT2 = 13                # k tiles for matmul2: 11.72 h-tiles + exp/bias tile
K2P = KT2 * 128
W1SCALE = 64.0          # power of two; moves W1 into e4m3 normal range

F32 = mybir.dt.float32
F16 = mybir.dt.float16
F8 = mybir.dt.float8e4  # ml_dtypes.float8_e4m3

# k-tile DMA chunking per block (even sizes; final chunks small so the
# last-byte -> last-matmul catch-up is short)
W1_CHUNKS = [[40, 40]] * (NBLK - 1) + [[40, 26, 10, 4]]


def _build_nc():
    nc = bacc.Bacc(
        "TRN2",
        target_bir_lowering=False,
        debug=False,
        enable_asserts=False,
        num_devices=NCORES,
    )

    # External I/O, all in SBUF-image layout (128, free)
    w1_d = nc.dram_tensor("w1_img", [128, KT1 * HIDP], F8, kind="ExternalInput")
    gos_d = nc.dram_tensor("gos_img", [128, KT1 * B], F8, kind="ExternalInput")
    w2_d = nc.dram_tensor("w2_img", [128, KT2 * CD], F16, kind="ExternalInput")
    exp_d = nc.dram_tensor("exp_img", [128, B], F16, kind="ExternalInput")
    mt_d = nc.dram_tensor("mt_img", [128, 2 * C], F16, kind="ExternalInput")
    out_d = nc.dram_tensor("out_img", [128, 2 * B], F32, kind="ExternalOutput")

    with tile.TileContext(nc) as tc:
        with (
            tc.tile_pool(name="big", bufs=1) as pp,
            tc.tile_pool(name="small", bufs=1) as sp,
            tc.tile_pool(name="ph", bufs=3, space="PSUM") as php,
            tc.tile_pool(name="pt", bufs=2, space="PSUM") as ptp,
            tc.tile_pool(name="pf", bufs=1, space="PSUM") as pfp,
        ):
            # identity for PE transposes (built on gpsimd, no DMA)
            ident = sp.tile([B, B], F16, tag="ident")
            make_identity(nc, ident[:, :])

            # --- small loads on the scalar HWDGE ring (Q10), off the W1 path
            exp_sb = sp.tile([128, B], F16, tag="exp")
            nc.scalar.dma_start(out=exp_sb[:, :], in_=exp_d[:, :])
            w2_sb = sp.tile([128, KT2 * CD], F16, tag="w2")
            nc.scalar.dma_start(out=w2_sb[:, :], in_=w2_d[:, :])
            mt_sb = pp.tile([128, 2 * C], F16, tag="mt")
            cm_sb = sp.tile([128, 2], F32, tag="cm")
            for cb in range(2):
                sl = slice(cb * C, (cb + 1) * C)
                nc.scalar.dma_start(out=mt_sb[:, sl], in_=mt_d[:, sl])
                nc.vector.reduce_max(
                    cm_sb[:, cb : cb + 1], mt_sb[:, sl], axis=mybir.AxisListType.X
                )

            # --- gos (stationary operand) first on the sync ring (Q1)
            gos_sb = pp.tile([128, KT1 * B], F8, tag="gos")
            nc.sync.dma_start(out=gos_sb[:, :], in_=gos_d[:, :])

            # --- W1 streamed block-major; matmul1 consumes chunk by chunk
            w1_sb = pp.tile([128, KT1 * HIDP], F8, tag="w1")
            h_sb = sp.tile([B, HIDP], F16, tag="h")
            xT_sb = sp.tile([128, KT2 * B], F16, tag="xT")
            # k-tile 11 of x.T covers h rows 1408..1499 only; zero the unused
            # partitions once so the (zero-W2-row) matmul2 products stay
            # finite (partition base must be 32-aligned; the transpose copy
            # later overwrites rows 64..91)
            nc.vector.memset(xT_sb[64:128, 11 * B : 12 * B], 0.0)
            psum_f = [
                pfp.tile([128, B], F32, tag=f"pf{cb}", name=f"pf{cb}")
                for cb in range(2)
            ]

            def mm2(cb, kt, start, stop):
                w2sl = w2_sb[:, kt * CD + cb * 128 : kt * CD + cb * 128 + 128]
                nc.tensor.matmul(
                    psum_f[cb][:, :],
                    lhsT=w2sl,
                    rhs=(exp_sb[:, :] if kt == KT2 - 1 else xT_sb[:, kt * B : (kt + 1) * B]),
                    start=start,
                    stop=stop,
                )

            # all W1 DMAs up front (one queue, in stream order)
            for nb in range(NBLK):
                base = BLK_OFF[nb] * KT1
                w = BLK_W[nb]
                ci = 0
                for ch in W1_CHUNKS[nb]:
                    sl = slice(base + ci * w, base + (ci + ch) * w)
                    nc.sync.dma_start(out=w1_sb[:, sl], in_=w1_d[:, sl])
                    ci += ch

            def mm1_block(nb):
                base = BLK_OFF[nb] * KT1
                w = BLK_W[nb]
                psh = php.tile([B, w], F32, tag="ph", name="ph")
                for t in range(KT1 // 2):
                    nc.tensor.matmul(
                        psh[:, :],
                        lhsT=gos_sb[:, 2 * t * B : (2 * t + 2) * B].rearrange(
                            "p (k b) -> p k b", k=2
                        ),
                        rhs=w1_sb[
                            :, base + 2 * t * w : base + (2 * t + 2) * w
                        ].rearrange("p (k f) -> p k f", k=2),
                        start=(t == 0),
                        stop=(t == KT1 // 2 - 1),
                        perf_mode=mybir.MatmulPerfMode.DoubleRow,
                    )
                return psh

            def block_tail(nb, psh):
                off, w = BLK_OFF[nb], BLK_W[nb]
                # gelu undoes the host-side W1 scaling; erf gelu
                nc.scalar.activation(
                    h_sb[:, off : off + w],
                    psh[:, :],
                    mybir.ActivationFunctionType.Gelu,
                    scale=1.0 / W1SCALE,
                )
                # transpose the 2 fresh h k-tiles and feed matmul2
                for q in range(2):
                    kt = nb * 2 + q
                    tw = min(128, HIDP - kt * 128)  # k-tile 11 is 92 rows
                    pt = ptp.tile([128, B], F16, tag="pt")
                    nc.tensor.transpose(
                        pt[0:tw, :], h_sb[:, kt * 128 : kt * 128 + tw], ident[:, :]
                    )
                    nc.vector.tensor_copy(
                        xT_sb[0:tw, kt * B : (kt + 1) * B], pt[0:tw, :]
                    )
                for cb in range(2):
                    for q in range(2):
                        kt = nb * 2 + q
                        mm2(cb, kt, start=False, stop=(nb == NBLK - 1 and q == 1))

            # PE issue order: block nb's gelu/transpose/mm2 tail is queued
            # AFTER block nb+1's matmul1 stream, so the in-order PE queue
            # never stalls on the ACT engine mid-stream.
            prev = mm1_block(0)
            for cb in range(2):
                # exp/bias k-tile opens the psum_f groups; operands arrive
                # early on Q10, long before the first h transposes.
                mm2(cb, KT2 - 1, start=True, stop=False)
            for nb in range(1, NBLK):
                cur = mm1_block(nb)
                block_tail(nb - 1, prev)
                prev = cur
            block_tail(NBLK - 1, prev)

            # sigmoid(pre) on ACT, then the colmax scale on DVE (keeps the two
            # stages on different engines); outputs split over both rings
            f_sb = sp.tile([128, 2 * B], F32, tag="f")
            o_sb = sp.tile([128, 2 * B], F32, tag="o")
            for cb in range(2):
                nc.scalar.activation(
                    f_sb[:, cb * B : (cb + 1) * B],
                    psum_f[cb][:, :],
                    mybir.ActivationFunctionType.Sigmoid,
                )
                nc.vector.tensor_scalar_mul(
                    o_sb[:, cb * B : (cb + 1) * B],
                    f_sb[:, cb * B : (cb + 1) * B],
                    cm_sb[:, cb : cb + 1],
                )
                (nc.sync if cb == 0 else nc.scalar).dma_start(
                    out=out_d[:, cb * B : (cb + 1) * B],
                    in_=o_sb[:, cb * B : (cb + 1) * B],
                )

    nc.compile()
    return nc


_NC_CACHE = None


def _get_nc():
    global _NC_CACHE
    if _NC_CACHE is None:
        _NC_CACHE = _build_nc()
    return _NC_CACHE


def _prep_inputs(gos, exp_x, W1, b1, W2, b2, hpo_matrix):
    f = np.float32
    gos = np.asarray(gos, f)
    exp_x = np.asarray(exp_x, f)
    W1 = np.asarray(W1, f)
    b1 = np.asarray(b1, f)
    W2 = np.asarray(W2, f)
    b2 = np.asarray(b2, f)
    M = np.asarray(hpo_matrix, f)
    f8 = ml_dtypes.float8_e4m3

    # W1 padded to K1P rows; bias row at K1P-1 pairs with the gos ones-row
    W1p = np.zeros((K1P, HIDP), f)
    W1p[:IN] = W1
    W1p[K1P - 1] = b1
    W1p8 = (W1p * W1SCALE).astype(f8)
    w1_img = np.concatenate(
        [
            W1p8[:, o : o + w]
            .reshape(KT1, 128, w)
            .transpose(1, 0, 2)
            .reshape(128, KT1 * w)
            for o, w in zip(BLK_OFF, BLK_W)
        ],
        axis=1,
    )
    w1_img = np.ascontiguousarray(w1_img)

    # gos.T padded to K1P rows with the ones-row last (b1 fold)
    gosT = np.zeros((K1P, B), f)
    gosT[:IN] = gos.T
    gosT[K1P - 1] = 1.0
    gos_img = np.ascontiguousarray(
        gosT.astype(f8).reshape(KT1, 128, B).transpose(1, 0, 2).reshape(128, KT1 * B)
    )

    # exp/bias k-tile of x.T: rows 0..52 exp.T, row 53 ones (b2 fold)
    exp_img = np.zeros((128, B), np.float16)
    exp_img[:EXP] = exp_x.T.astype(np.float16)
    exp_img[EXP] = 1.0

    # W2 rows remapped to x.T layout: h in rows 0..1499 (k-tiles 0..11, the
    # last one ragged), exp in k-tile 12 rows 1536..1588, b2 row at 1589
    W2p = np.zeros((K2P, C), f)
    W2p[:HID] = W2[:HID]
    W2p[12 * 128 : 12 * 128 + EXP] = W2[HID:]
    W2p[12 * 128 + EXP] = b2
    W2p16 = W2p.astype(np.float16)

    in_maps = []
    for c in range(NCORES):
        c0 = CD * c
        w2_img = np.ascontiguousarray(
            W2p16[:, c0 : c0 + CD]
            .reshape(KT2, 128, CD)
            .transpose(1, 0, 2)
            .reshape(128, KT2 * CD)
        )
        mt = M[:, c0 : c0 + CD].T.astype(np.float16)  # (256, 2048)
        mt_img = np.ascontiguousarray(np.concatenate([mt[:128], mt[128:]], axis=1))
        in_maps.append(
            {
                "w1_img": w1_img,
                "gos_img": gos_img,
                "w2_img": w2_img,
                "exp_img": exp_img,
                "mt_img": mt_img,
            }
        )
    return in_maps


def _assemble_output(results):
    cols = []
    for r in results:
        o = r["out_img"]  # (128, 2B): [p, cb*B + b] = out[b, c0 + cb*128 + p]
        chunk = o.reshape(128, 2, B).transpose(1, 0, 2).reshape(CD, B)
        cols.append(chunk.T)  # (B, CD)
    return np.ascontiguousarray(np.concatenate(cols, axis=1))


def kernel(gos, exp_x, W1, b1, W2, b2, hpo_matrix, **kw):
    nc = _get_nc()
    in_maps = _prep_inputs(gos, exp_x, W1, b1, W2, b2, hpo_matrix)
    res = run_bass_kernel_spmd(nc, in_maps, core_ids=list(range(NCORES)))
    return _assemble_output(res.results)
